# revision 1
# baseline (speedup 1.0000x reference)
"""Two-layer GCN (GCNConv x2, PyG-style symmetric normalization) on 8 trn2
NeuronCores.

Strategy (SWDGE-minimal, piece-pipelined):
  - Nodes sharded into 8 contiguous buckets (12800 rows padded/core, 100
    blocks of 128).  Core c owns bucket c and aggregates all edges whose
    destination falls in its bucket.
  - Layer 1 uses NO device gather: aggregation commutes with the dense
    transform, so we aggregate raw x first and apply W1 after.  The host
    expands x into destination-sorted edge order (pure index plumbing,
    np.take) and ships it as a partition-major bf16 plane that streams
    sequentially from HBM.  The per-edge 1/sqrt(deg_src) weight is fused
    into the one-hot mask build: mask = (iota==rel) * rsqrt(deg_e) (one
    DVE tensor_scalar with two per-partition scalar operands).
  - Aggregation runs transposed on the TensorEngine: psumT[ch,slot] +=
    data_tile[e,ch].T @ mask[e,slot] (stationary=data, moving=mask), so
    the post-aggregation transform is a single matmul lhsT=aggT rhs=W.
    Bias is pre-seeded into PSUM as sqrt(deg)[d]*b[ch] (K=4 selector
    matmul) so eviction is one fused (psum*dis, relu) op.  PSUM discipline:
    exactly ONE accumulation group per 2KB bank (bank-wide opener matmul,
    one stop per bank) — interleaved per-strip groups corrupt the bank.
  - Layer 1 eviction writes the layer-2 gather table pre-scaled:
    t2 = dis * relu(h1), so layer-2 masks are pure one-hot and the
    layer-2 self loop is one identity matmul per block from local SBUF.
  - Layer 2 is the only SWDGE user (the hard bottleneck: descriptor
    generation costs ~8ns/row, serial on the GpSimd Q7 cluster).  Edges
    are sorted by (dst block, src piece) and gathered per (piece, super=8
    blocks) batch with int16 indices into a 25600-row table piece.
  - Piece-major pipelining: each piece AllGather + its gathers are
    emitted before the next piece, so gathers start ~25% into layer 1
    and overlap it; per-piece PSUM partials spill to SBUF (bf16) and are
    re-injected as the next pass\'s bank-opener matmul, keeping staging
    and PSUM footprints bounded.

Host-side work is index plumbing only: bucketing/sorting edges, expansion
of x rows (np.take) + bf16 layout, degree counts, index/mask-value planes.
All floating-point math (rsqrt, matmuls, scaling, bias, relu) runs on
device.
"""

import math
import os

import numpy as np

DBG_NO_COLL = os.environ.get("GCN_NO_COLL", "0") == "1"
DBG_L1_OUT = os.environ.get("GCN_L1_OUT", "0") == "1"

CFG_FULL = dict(N=100000, E=1600000, CIN=128, CHID=128, COUT=64)

NCORES = 8
PIECES = 4  # table pieces / AllGather splits
SUPER = 8  # dst blocks per gather batch group


def _derive(cfg):
    n = cfg["N"]
    bucket = n // NCORES
    assert bucket * NCORES == n
    blocks = math.ceil(bucket / 128)
    blocks = math.ceil(blocks / PIECES) * PIECES
    shard = blocks * 128
    qrows = shard // PIECES  # rows per piece per core
    chunk = qrows * NCORES  # rows of one assembled table piece
    assert chunk <= 32600, chunk  # int16 gather index limit
    supers = [SUPER] * (blocks // SUPER)
    if blocks % SUPER:
        supers.append(blocks % SUPER)
    bpp = blocks // PIECES
    return dict(bucket=bucket, blocks=blocks, shard=shard, qrows=qrows,
                chunk=chunk, supers=supers, bpp=bpp)


def _prep_l1(x, edge_index, deg, cfg, d):
    """Host expansion of x into dst-sorted edge order (incl self loops)."""
    import ml_dtypes
    n = cfg["N"]
    bucket, blocks = d["bucket"], d["blocks"]
    row = edge_index[0].astype(np.int64)
    col = edge_index[1].astype(np.int64)
    loop = np.arange(n, dtype=np.int64)
    r1 = np.concatenate([row, loop])
    c1 = np.concatenate([col, loop])

    c_dst = c1 // bucket
    d_l = c1 - c_dst * bucket
    blk = d_l // 128
    rel = (d_l % 128).astype(np.float32)

    cnt1 = np.zeros((NCORES, blocks), np.int64)
    for c in range(NCORES):
        m = c_dst == c
        cnt1[c] = np.bincount(blk[m], minlength=blocks)
    tiles1 = np.ceil(cnt1.max(axis=0) / 128).astype(np.int64)  # [blocks]
    off1 = np.concatenate([[0], np.cumsum(tiles1)])
    tot1 = int(off1[-1])

    per_core = []
    for c in range(NCORES):
        m = c_dst == c
        bm = blk[m]
        sort = np.argsort(bm, kind="stable")
        bs = bm[sort]
        srcs = r1[m][sort]
        rels = rel[m][sort]
        grp_start = np.searchsorted(bs, bs)
        within = np.arange(bs.size) - grp_start
        slots = off1[bs] * 128 + within
        src_flat = np.full(tot1 * 128, -1, np.int64)
        rel_flat = np.full(tot1 * 128, -1.0, np.float32)
        src_flat[slots] = srcs
        rel_flat[slots] = rels
        valid = src_flat >= 0
        xe = np.zeros((tot1 * 128, cfg["CIN"]), np.float32)
        xe[valid] = x[src_flat[valid]]
        dege = np.ones(tot1 * 128, np.float32)
        dege[valid] = deg[src_flat[valid]]
        # partition-major plane: [128, tot1, CIN] where lane p of tile t is
        # expanded row t*128+p
        xpm = np.ascontiguousarray(
            xe.reshape(tot1, 128, cfg["CIN"]).transpose(1, 0, 2)
            .reshape(128, tot1 * cfg["CIN"]).astype(ml_dtypes.bfloat16))
        rel1 = np.ascontiguousarray(rel_flat.reshape(tot1, 128).T)
        deg1 = np.ascontiguousarray(dege.reshape(tot1, 128).T)
        per_core.append(dict(x_exp=xpm, rel1=rel1, deg1=deg1))
    return dict(tiles1=tiles1, off1=off1, tot1=tot1), per_core


def _prep_l2(edge_index, cfg, d):
    """Bucket & sort edges (no self loops), per-core gather/mask planes."""
    bucket, blocks, qrows = d["bucket"], d["blocks"], d["qrows"]
    row = edge_index[0].astype(np.int64)
    col = edge_index[1].astype(np.int64)

    c_dst = col // bucket
    d_l = col - c_dst * bucket
    blk = d_l // 128
    rel = (d_l % 128).astype(np.float32)
    c_src = row // bucket
    r_l = row - c_src * bucket
    q = r_l // qrows
    ric = (c_src * qrows + r_l % qrows).astype(np.int64)  # row in chunk q

    nbq = blocks * PIECES
    key_bq = blk * PIECES + q
    counts = np.zeros((NCORES, nbq), np.int64)
    for c in range(NCORES):
        m = c_dst == c
        counts[c] = np.bincount(key_bq[m], minlength=nbq)
    tiles_bq = np.ceil(counts.max(axis=0) / 128).astype(np.int64)  # [nbq]

    # tile schedule in program order: piece-major (piece, super, block) so
    # piece-q gathers can all run as soon as AllGather q lands
    order_bq = []
    supers = d["supers"]
    batches = []  # tiles per (piece, super) gather batch
    for qq in range(PIECES):
        b0 = 0
        for g in supers:
            nt = 0
            for b in range(b0, b0 + g):
                order_bq.append((b, qq))
                nt += int(tiles_bq[b * PIECES + qq])
            batches.append(nt)
            b0 += g
    tot2 = int(tiles_bq.sum())
    assert sum(batches) == tot2 and tot2 > 0

    off_bq = np.zeros(nbq, np.int64)
    acc = 0
    for (b, qq) in order_bq:
        off_bq[b * PIECES + qq] = acc
        acc += int(tiles_bq[b * PIECES + qq])

    per_core = []
    for c in range(NCORES):
        m = c_dst == c
        okey = (blk[m] * PIECES + q[m]).astype(np.int64)
        sort = np.argsort(okey, kind="stable")
        okey_s = okey[sort]
        e_rel = rel[m][sort]
        e_ric = ric[m][sort]
        slot_base = off_bq[okey_s] * 128
        grp_start = np.searchsorted(okey_s, okey_s)
        within = np.arange(okey_s.size) - grp_start
        slots = slot_base + within
        idx_flat = np.zeros(tot2 * 128, np.int16)
        rel_flat = np.full(tot2 * 128, -1.0, np.float32)
        idx_flat[slots] = e_ric.astype(np.int16)
        rel_flat[slots] = e_rel
        idx16 = idx_flat.reshape(tot2 * 8, 16).T  # [16, tiles*8]
        idx_plane = np.tile(idx16, (8, 1)).copy()
        rel_plane = np.ascontiguousarray(
            rel_flat.reshape(tot2, 128).T)  # [128, tot2]
        per_core.append(dict(idx_plane=idx_plane, rel2=rel_plane))

    meta2 = dict(tiles_bq=tiles_bq, batches=batches, tot2=tot2,
                 supers=supers)
    return meta2, per_core


def _host_inputs(x, edge_index, W1, b1, W2, b2, cfg):
    d = _derive(cfg)
    bucket, blocks, shard = d["bucket"], d["blocks"], d["shard"]
    n = cfg["N"]
    chid, cout = cfg["CHID"], cfg["COUT"]

    col = edge_index[1].astype(np.int64)
    deg = (np.bincount(col, minlength=n) + 1).astype(np.float32)

    meta1, pc1 = _prep_l1(np.asarray(x, np.float32), edge_index, deg, cfg, d)
    meta2, pc2 = _prep_l2(edge_index, cfg, d)

    w1 = np.ascontiguousarray(np.asarray(W1, np.float32))
    w2 = np.ascontiguousarray(np.asarray(W2, np.float32))
    b1r = np.asarray(b1, np.float32).reshape(1, chid)
    b2r = np.asarray(b2, np.float32).reshape(1, cout)
    iota = np.ascontiguousarray(
        np.broadcast_to(np.arange(128, dtype=np.float32)[None, :],
                        (128, 128)))
    eye = np.eye(128, dtype=np.float32)

    in_maps = []
    for c in range(NCORES):
        degs = np.ones(shard, np.float32)
        degs[:bucket] = deg[c * bucket:(c + 1) * bucket]
        deg_pm = np.ascontiguousarray(degs.reshape(blocks, 128).T)
        # [4, (blocks//4)*128]: [k, g*128+p] = deg[(4g+k)*128+p]
        deg_b4 = np.ascontiguousarray(
            degs.reshape(blocks // 4, 4, 128).transpose(1, 0, 2)
            .reshape(4, -1))
        b1sel = np.zeros((4, 4 * chid), np.float32)
        b2sel = np.zeros((4, 4 * cout), np.float32)
        for k in range(4):
            b1sel[k, k * chid:(k + 1) * chid] = b1r[0]
            b2sel[k, k * cout:(k + 1) * cout] = b2r[0]
        in_maps.append({
            "x_exp": pc1[c]["x_exp"], "rel1": pc1[c]["rel1"],
            "deg1": pc1[c]["deg1"],
            "idx_plane": pc2[c]["idx_plane"], "rel2": pc2[c]["rel2"],
            "deg_pm": deg_pm, "deg_b4": deg_b4,
            "w1": w1, "w2": w2, "b1sel": b1sel, "b2sel": b2sel,
            "iota": iota, "eye": eye,
        })
    meta = dict(d=d, m1=meta1, m2=meta2)
    return meta, in_maps


def _build_program(cfg, meta):
    import concourse.bacc as bacc
    import concourse.mybir as mybir
    from concourse import tile

    d = meta["d"]
    blocks, shard, qrows, chunk, bpp = (d["blocks"], d["shard"], d["qrows"],
                                        d["chunk"], d["bpp"])
    supers = d["supers"]
    tiles1 = meta["m1"]["tiles1"]
    off1 = meta["m1"]["off1"]
    tot1 = meta["m1"]["tot1"]
    tiles_bq = meta["m2"]["tiles_bq"]
    tot2 = meta["m2"]["tot2"]
    batches = meta["m2"]["batches"]
    cin, chid, cout = cfg["CIN"], cfg["CHID"], cfg["COUT"]
    nbmax = max(batches)

    bf16 = mybir.dt.bfloat16
    f32 = mybir.dt.float32
    i16 = mybir.dt.int16
    mult = mybir.AluOpType.mult
    amax = mybir.AluOpType.max
    iseq = mybir.AluOpType.is_equal

    nc = bacc.Bacc("TRN2", target_bir_lowering=False, debug=False,
                   num_devices=NCORES)

    x_exp_t = nc.dram_tensor("x_exp", [128, tot1 * cin], bf16,
                             kind="ExternalInput")
    rel1_t = nc.dram_tensor("rel1", [128, tot1], f32, kind="ExternalInput")
    deg1_t = nc.dram_tensor("deg1", [128, tot1], f32, kind="ExternalInput")
    idxp_t = nc.dram_tensor("idx_plane", [128, tot2 * 8], i16,
                            kind="ExternalInput")
    rel2_t = nc.dram_tensor("rel2", [128, tot2], f32, kind="ExternalInput")
    deg_pm_t = nc.dram_tensor("deg_pm", [128, blocks], f32,
                              kind="ExternalInput")
    deg_b4_t = nc.dram_tensor("deg_b4", [4, (blocks // 4) * 128], f32,
                              kind="ExternalInput")
    w1_t = nc.dram_tensor("w1", [cin, chid], f32, kind="ExternalInput")
    w2_t = nc.dram_tensor("w2", [chid, cout], f32, kind="ExternalInput")
    b1sel_t = nc.dram_tensor("b1sel", [4, 4 * chid], f32,
                             kind="ExternalInput")
    b2sel_t = nc.dram_tensor("b2sel", [4, 4 * cout], f32,
                             kind="ExternalInput")
    iota_t = nc.dram_tensor("iota", [128, 128], f32, kind="ExternalInput")
    eye_t = nc.dram_tensor("eye", [128, 128], f32, kind="ExternalInput")
    out_t = nc.dram_tensor("out", [shard, cout], f32, kind="ExternalOutput")

    with tile.TileContext(nc) as tc:
        with (
            tc.tile_pool(name="dram", bufs=1, space="DRAM") as dram,
            tc.tile_pool(name="const", bufs=1) as cp,
            tc.tile_pool(name="xstage", bufs=3) as xsp,
            tc.tile_pool(name="stage", bufs=4) as stp,
            tc.tile_pool(name="masks", bufs=12) as mp,
            tc.tile_pool(name="work", bufs=6) as wp,
            tc.tile_pool(name="outp", bufs=4) as op_,
            tc.tile_pool(name="pagg", bufs=4, space="PSUM") as pagg,
            tc.tile_pool(name="pfin", bufs=2, space="PSUM") as pfin,
        ):
            # ---- DRAM scratch ----
            bounce = [dram.tile([qrows, chid], bf16, name=f"bo_{j}",
                                tag=f"bo_{j}") for j in range(PIECES)]
            tab2 = [dram.tile([chunk, chid], bf16, name=f"t2_{j}",
                              tag=f"t2_{j}")
                    for j in range(PIECES)]

            # ---- constants ----
            iota_sb = cp.tile([128, 128], bf16)
            nc.gpsimd.dma_start(iota_sb[:], iota_t[:])  # cast f32->bf16
            eye_sb = cp.tile([128, 128], bf16)
            nc.gpsimd.dma_start(eye_sb[:], eye_t[:])
            w1_sb = cp.tile([cin, chid], bf16)
            nc.gpsimd.dma_start(w1_sb[:], w1_t[:])
            w2_sb = cp.tile([chid, cout], bf16)
            nc.gpsimd.dma_start(w2_sb[:], w2_t[:])
            b1_sb = cp.tile([4, 4 * chid], f32)
            nc.sync.dma_start(b1_sb[:], b1sel_t[:])
            b2_sb = cp.tile([4, 4 * cout], f32)
            nc.sync.dma_start(b2_sb[:], b2sel_t[:])
            idxp_sb = cp.tile([128, tot2 * 8], i16)
            nc.sync.dma_start(idxp_sb[:], idxp_t[:])
            rel1_sb = cp.tile([128, tot1], f32)
            nc.sync.dma_start(rel1_sb[:], rel1_t[:])
            deg1_sb = cp.tile([128, tot1], f32)
            nc.sync.dma_start(deg1_sb[:], deg1_t[:])
            rel2_sb = cp.tile([128, tot2], f32)
            nc.sync.dma_start(rel2_sb[:], rel2_t[:])
            deg_pm = cp.tile([128, blocks], f32)
            nc.sync.dma_start(deg_pm[:], deg_pm_t[:])
            deg_b4 = cp.tile([4, (blocks // 4) * 128], f32)
            nc.sync.dma_start(deg_b4[:], deg_b4_t[:])

            # device rsqrt of all degree planes
            sq1 = cp.tile([128, tot1], f32)
            nc.scalar.sqrt(sq1[:], deg1_sb[:])
            dis1 = cp.tile([128, tot1], f32)
            nc.vector.reciprocal(dis1[:], sq1[:])
            invd_pm = cp.tile([128, blocks], f32)
            nc.scalar.sqrt(invd_pm[:], deg_pm[:])
            dis_pm = cp.tile([128, blocks], f32)
            nc.vector.reciprocal(dis_pm[:], invd_pm[:])
            invd_b4 = cp.tile([4, (blocks // 4) * 128], f32)
            nc.scalar.sqrt(invd_b4[:], deg_b4[:])

            # t2 shard rows (this core's bucket, pre-scaled by dis)
            g2s = cp.tile([128, blocks * chid], bf16)
            nc.vector.memset(g2s[:], 0.0)
            zrow = cp.tile([1, 512], f32)
            nc.vector.memset(zrow[:], 0.0)

            def seed_bias(psum, b, bias_sb, w):
                # psum[slot, ch] = sqrt(deg_b)[slot] * bias[ch] via K=4
                # matmul: invd_b4 column group of b with selector row b%4
                nc.tensor.matmul(
                    psum,
                    invd_b4[:, (b // 4) * 128:(b // 4) * 128 + 128],
                    bias_sb[:, (b % 4) * w:(b % 4) * w + w],
                    start=True, stop=False)

            # ========= layer 1 + layer 2 wavefront emission =========
            # L2 (piece, super) work is emitted as soon as its deps exist
            # (g2s blocks of the super + AllGathered table piece), so the
            # PE consumes gather batches while layer 1 is still running
            # and the staging pool keeps recycling for the GpSimd queue.
            part = cp.tile([128, blocks * 128], bf16)
            x3 = x_exp_t[:].rearrange("p (t c) -> p t c", c=cin)
            nsup = len(supers)
            sup_b0 = [sum(supers[:i]) for i in range(nsup)]
            batch_prefix = [0]
            for nb_ in batches:
                batch_prefix.append(batch_prefix[-1] + nb_)
            st_ctr = [0]

            def emit_l2_item(qq, si):
                g = supers[si]
                b0 = sup_b0[si]
                bidx = qq * nsup + si
                cursor = int(batch_prefix[bidx])
                nb = int(batches[bidx])
                banks2 = [pagg.tile([128, 512], f32, tag="bank2",
                                    bufs=4, name="p2b")
                          for _ in range(g // 4)]

                def pacc(bi):
                    return banks2[bi // 4][:, (bi % 4) * 128:
                                           (bi % 4) * 128 + 128]

                # ONE accumulation group per PSUM bank: a bank-wide
                # opener matmul (zeros on pass 0, previous partial
                # re-injected via identity otherwise), one stop/bank.
                seq = []
                if qq == 0:
                    seq += [("self", bi) for bi in range(g)]
                for bi in range(g):
                    ntq = int(tiles_bq[(b0 + bi) * PIECES + qq])
                    for t in range(ntq):
                        seq.append(("edge", bi, t))
                last_per_bank = {k: ("opener", k) for k in range(g // 4)}
                for item in seq:
                    last_per_bank[item[1] // 4] = item
                for k in range(g // 4):
                    stop_k = last_per_bank[k] == ("opener", k)
                    if qq == 0:
                        nc.tensor.matmul(banks2[k][:], zrow[:, 0:128],
                                         zrow[:], start=True, stop=stop_k)
                    else:
                        nc.tensor.matmul(
                            banks2[k][:], eye_sb[:],
                            part[:, (b0 + k * 4) * 128:
                                 (b0 + k * 4 + 4) * 128],
                            start=True, stop=stop_k)
                if qq == 0:
                    # self loops: p2T[ch,slot] += t2_local.T (t2 already
                    # carries one dis; eviction supplies the second)
                    for bi in range(g):
                        b = b0 + bi
                        nc.tensor.matmul(
                            pacc(bi), g2s[:, b * chid:(b + 1) * chid],
                            eye_sb[:], start=False,
                            stop=(last_per_bank[bi // 4] == ("self", bi)))
                if nb > 0:
                    st = stp.tile([128, nbmax, 128], bf16, tag="st",
                                  name="st")
                    if st_ctr[0] < 4:
                        nc.vector.memset(st[:], 0.0)
                    st_ctr[0] += 1
                    nc.gpsimd.dma_gather(
                        st[:, :nb, :], tab2[qq][:],
                        idxp_sb[:, cursor * 8:(cursor + nb) * 8],
                        nb * 128, nb * 128, 128,
                        single_packet=False)
                    t_local = 0
                    for bi in range(g):
                        b = b0 + bi
                        ntq = int(tiles_bq[b * PIECES + qq])
                        for t in range(ntq):
                            gcol = cursor + t_local
                            mk = mp.tile([128, 128], bf16, tag="mk",
                                         name="mk2")
                            nc.vector.tensor_scalar(
                                mk[:], iota_sb[:],
                                rel2_sb[:, gcol:gcol + 1], None, iseq)
                            stop = (last_per_bank[bi // 4] ==
                                    ("edge", bi, t))
                            nc.tensor.matmul(
                                pacc(bi),
                                st[:, t_local:t_local + 1, :].squeeze(),
                                mk[:], start=False, stop=stop)
                            t_local += 1
                if qq < PIECES - 1:
                    # spill partial aggregate for the next piece pass
                    for bi in range(g):
                        b = b0 + bi
                        nc.vector.tensor_copy(
                            part[:, b * 128:(b + 1) * 128], pacc(bi))
                else:
                    # final: transform + bias + scale, write out
                    fin2 = None
                    for bi in range(g):
                        b = b0 + bi
                        if bi % 4 == 0:
                            fin2 = pfin.tile([128, 512], f32, tag="fin",
                                             name="pob")
                        a2T = wp.tile([128, 128], bf16, tag="aggT",
                                      name="a2T")
                        nc.vector.tensor_copy(a2T[:], pacc(bi))
                        po = fin2[:, (bi % 4) * 128:(bi % 4) * 128 + cout]
                        seed_bias(po, b, b2_sb, cout)
                        nc.tensor.matmul(po, a2T[:], w2_sb[:],
                                         start=False, stop=True)
                        ob = op_.tile([128, cout], f32, tag="ob",
                                      name="ob")
                        nc.vector.tensor_scalar(
                            ob[:], po, dis_pm[:, b:b + 1], None, mult)
                        nc.sync.dma_start(
                            out_t[b * 128:(b + 1) * 128, :], ob[:])

            # ---- layer 1 with interleaved L2 emission ----
            for b4 in range(0, blocks, 4):
                bank = pagg.tile([128, 512], f32, tag="bank1",
                                 bufs=2, name="p1b")
                fin = pfin.tile([128, 512], f32, tag="fin", name="phb")
                for bi in range(4):
                    b = b4 + bi
                    nt = int(tiles1[b])
                    t0 = int(off1[b])
                    if nt > 0:
                        p1 = bank[:, bi * 128:bi * 128 + 128]
                        xs = xsp.tile([128, nt, cin], bf16, tag="xs",
                                      name="xs")
                        nc.sync.dma_start(xs[:], x3[:, t0:t0 + nt, :])
                        for t in range(nt):
                            mk = mp.tile([128, 128], bf16, tag="mk",
                                         name="mk")
                            nc.vector.tensor_scalar(
                                mk[:], iota_sb[:],
                                rel1_sb[:, t0 + t:t0 + t + 1],
                                dis1[:, t0 + t:t0 + t + 1], iseq, mult)
                            nc.tensor.matmul(p1, xs[:, t, :].squeeze(),
                                             mk[:], start=(t == 0),
                                             stop=(t == nt - 1))
                        aggT = wp.tile([128, 128], bf16, tag="aggT",
                                       name="aggT")
                        nc.vector.tensor_copy(aggT[:], p1)
                        ph = fin[:, bi * 128:bi * 128 + chid]
                        seed_bias(ph, b, b1_sb, chid)
                        nc.tensor.matmul(ph, aggT[:], w1_sb[:],
                                         start=False, stop=True)
                        h1r = wp.tile([128, chid], bf16, tag="h1r",
                                      name="h1r")
                        nc.vector.tensor_scalar(h1r[:], ph,
                                                dis_pm[:, b:b + 1], 0.0,
                                                mult, amax)
                        nc.vector.tensor_scalar(
                            g2s[:, b * chid:(b + 1) * chid], h1r[:],
                            dis_pm[:, b:b + 1], None, mult)
                    # piece boundary: export, AllGather, release L2 work
                    if (b + 1) % bpp == 0:
                        j = b // bpp
                        g2s3 = g2s[:].rearrange("p (bb c) -> p bb c",
                                                c=chid)
                        nc.sync.dma_start(
                            bounce[j][:].rearrange("(bb p) c -> p bb c",
                                                   p=128),
                            g2s3[:, j * bpp:(j + 1) * bpp, :])

            if DBG_L1_OUT:
                for b in range(blocks):
                    ob = op_.tile([128, cout], f32, tag="ob", name="obd")
                    nc.vector.tensor_copy(
                        ob[:], g2s[:, b * chid:b * chid + cout])
                    nc.sync.dma_start(out_t[b * 128:(b + 1) * 128, :],
                                      ob[:])

            # ---- layer 2: piece-major passes (AllGather at pass start) ----
            for qq in (range(PIECES) if not DBG_L1_OUT else []):
                if DBG_NO_COLL:
                    nc.sync.dma_start(tab2[qq][0:qrows, :], bounce[qq][:])
                else:
                    nc.gpsimd.collective_compute(
                        "AllGather", mybir.AluOpType.bypass,
                        replica_groups=[list(range(NCORES))],
                        ins=[bounce[qq].opt()], outs=[tab2[qq].opt()])
                for si in range(nsup):
                    emit_l2_item(qq, si)

    nc.compile()
    return nc


def run_config(inputs, cfg, run=None):
    from concourse.bass_utils import run_bass_kernel_spmd

    x = np.asarray(inputs["x"], np.float32)
    edge_index = np.asarray(inputs["edge_index"])
    meta, in_maps = _host_inputs(
        x, edge_index, inputs["W1"], inputs["b1"], inputs["W2"],
        inputs["b2"], cfg)
    nc = _build_program(cfg, meta)
    if run is None:
        def run(nc, in_maps):
            return run_bass_kernel_spmd(
                nc, in_maps, list(range(NCORES))).results
    results = run(nc, in_maps)
    bucket = _derive(cfg)["bucket"]
    out = np.concatenate(
        [results[c]["out"][:bucket] for c in range(NCORES)], axis=0)
    return np.ascontiguousarray(out.astype(np.float32))


def kernel(**inputs):
    return run_config(inputs, CFG_FULL)



# revision 11
# speedup vs baseline: 1.0176x; 1.0176x over previous
"""Two-layer GCN (GCNConv x2, PyG-style symmetric normalization) on 8 trn2
NeuronCores.

Strategy (SWDGE-minimal, piece-pipelined):
  - Nodes sharded into 8 contiguous buckets (12800 rows padded/core, 100
    blocks of 128).  Core c owns bucket c and aggregates all edges whose
    destination falls in its bucket.
  - Layer 1 uses NO device gather: aggregation commutes with the dense
    transform, so we aggregate raw x first and apply W1 after.  The host
    expands x into destination-sorted edge order (pure index plumbing,
    np.take) and ships it as a partition-major bf16 plane that streams
    sequentially from HBM.  The per-edge 1/sqrt(deg_src) weight is fused
    into the one-hot mask build: mask = (iota==rel) * rsqrt(deg_e) (one
    DVE tensor_scalar with two per-partition scalar operands).
  - Aggregation runs transposed on the TensorEngine: psumT[ch,slot] +=
    data_tile[e,ch].T @ mask[e,slot] (stationary=data, moving=mask), so
    the post-aggregation transform is a single matmul lhsT=aggT rhs=W.
    Bias is pre-seeded into PSUM as sqrt(deg)[d]*b[ch] (K=4 selector
    matmul) so eviction is one fused (psum*dis, relu) op.  PSUM discipline:
    exactly ONE accumulation group per 2KB bank (bank-wide opener matmul,
    one stop per bank) — interleaved per-strip groups corrupt the bank.
  - Layer 1 eviction writes the layer-2 gather table pre-scaled:
    t2 = dis * relu(h1), so layer-2 masks are pure one-hot and the
    layer-2 self loop is one identity matmul per block from local SBUF.
  - Layer 2 is the only SWDGE user (the hard bottleneck: descriptor
    generation costs ~8ns/row, serial on the GpSimd Q7 cluster).  Edges
    are sorted by (dst block, src piece) and gathered per (piece, super=8
    blocks) batch with int16 indices into a 25600-row table piece.
  - Piece-major pipelining: each piece AllGather + its gathers are
    emitted before the next piece, so gathers start ~25% into layer 1
    and overlap it; per-piece PSUM partials spill to SBUF (bf16) and are
    re-injected as the next pass\'s bank-opener matmul, keeping staging
    and PSUM footprints bounded.

Host-side work is index plumbing only: bucketing/sorting edges, expansion
of x rows (np.take) + bf16 layout, degree counts, index/mask-value planes.
All floating-point math (rsqrt, matmuls, scaling, bias, relu) runs on
device.
"""

import math
import os

import numpy as np

DBG_NO_COLL = os.environ.get("GCN_NO_COLL", "0") == "1"
DBG_L1_OUT = os.environ.get("GCN_L1_OUT", "0") == "1"

CFG_FULL = dict(N=100000, E=1600000, CIN=128, CHID=128, COUT=64)

NCORES = 8
PIECES = 4  # table pieces / AllGather splits
SUPER = 8  # dst blocks per gather batch group
GATHER_QUEUES = 4  # SWDGE queues to spread gather drains across


def _derive(cfg):
    n = cfg["N"]
    bucket = n // NCORES
    assert bucket * NCORES == n
    blocks = math.ceil(bucket / 128)
    blocks = math.ceil(blocks / PIECES) * PIECES
    shard = blocks * 128
    qrows = shard // PIECES  # rows per piece per core
    chunk = qrows * NCORES  # rows of one assembled table piece
    assert chunk <= 32600, chunk  # int16 gather index limit
    supers = [SUPER] * (blocks // SUPER)
    if blocks % SUPER:
        supers.append(blocks % SUPER)
    bpp = blocks // PIECES
    return dict(bucket=bucket, blocks=blocks, shard=shard, qrows=qrows,
                chunk=chunk, supers=supers, bpp=bpp)


def _prep_l1(x, edge_index, deg, cfg, d):
    """Host expansion of x into dst-sorted edge order (incl self loops)."""
    import ml_dtypes
    n = cfg["N"]
    bucket, blocks = d["bucket"], d["blocks"]
    row = edge_index[0].astype(np.int64)
    col = edge_index[1].astype(np.int64)
    loop = np.arange(n, dtype=np.int64)
    r1 = np.concatenate([row, loop])
    c1 = np.concatenate([col, loop])

    c_dst = c1 // bucket
    d_l = c1 - c_dst * bucket
    blk = d_l // 128
    rel = (d_l % 128).astype(np.float32)

    cnt1 = np.zeros((NCORES, blocks), np.int64)
    for c in range(NCORES):
        m = c_dst == c
        cnt1[c] = np.bincount(blk[m], minlength=blocks)
    tiles1 = np.ceil(cnt1.max(axis=0) / 128).astype(np.int64)  # [blocks]
    off1 = np.concatenate([[0], np.cumsum(tiles1)])
    tot1 = int(off1[-1])

    per_core = []
    for c in range(NCORES):
        m = c_dst == c
        bm = blk[m]
        sort = np.argsort(bm, kind="stable")
        bs = bm[sort]
        srcs = r1[m][sort]
        rels = rel[m][sort]
        grp_start = np.searchsorted(bs, bs)
        within = np.arange(bs.size) - grp_start
        slots = off1[bs] * 128 + within
        src_flat = np.full(tot1 * 128, -1, np.int64)
        rel_flat = np.full(tot1 * 128, -1.0, np.float32)
        src_flat[slots] = srcs
        rel_flat[slots] = rels
        valid = src_flat >= 0
        xe = np.zeros((tot1 * 128, cfg["CIN"]), np.float32)
        xe[valid] = x[src_flat[valid]]
        dege = np.ones(tot1 * 128, np.float32)
        dege[valid] = deg[src_flat[valid]]
        # partition-major plane: [128, tot1, CIN] where lane p of tile t is
        # expanded row t*128+p
        xpm = np.ascontiguousarray(
            xe.reshape(tot1, 128, cfg["CIN"]).transpose(1, 0, 2)
            .reshape(128, tot1 * cfg["CIN"]).astype(ml_dtypes.bfloat16))
        rel1 = np.ascontiguousarray(rel_flat.reshape(tot1, 128).T)
        deg1 = np.ascontiguousarray(dege.reshape(tot1, 128).T)
        per_core.append(dict(x_exp=xpm, rel1=rel1, deg1=deg1))
    return dict(tiles1=tiles1, off1=off1, tot1=tot1), per_core


def _prep_l2(edge_index, cfg, d):
    """Bucket & sort edges (no self loops), per-core gather/mask planes."""
    bucket, blocks, qrows = d["bucket"], d["blocks"], d["qrows"]
    row = edge_index[0].astype(np.int64)
    col = edge_index[1].astype(np.int64)

    c_dst = col // bucket
    d_l = col - c_dst * bucket
    blk = d_l // 128
    rel = (d_l % 128).astype(np.float32)
    c_src = row // bucket
    r_l = row - c_src * bucket
    q = r_l // qrows
    ric = (c_src * qrows + r_l % qrows).astype(np.int64)  # row in chunk q

    nbq = blocks * PIECES
    key_bq = blk * PIECES + q
    counts = np.zeros((NCORES, nbq), np.int64)
    for c in range(NCORES):
        m = c_dst == c
        counts[c] = np.bincount(key_bq[m], minlength=nbq)
    tiles_bq = np.ceil(counts.max(axis=0) / 128).astype(np.int64)  # [nbq]

    # tile schedule in program order: piece-major (piece, super, block) so
    # piece-q gathers can all run as soon as AllGather q lands
    order_bq = []
    supers = d["supers"]
    batches = []  # tiles per (piece, super) gather batch
    for qq in range(PIECES):
        b0 = 0
        for g in supers:
            nt = 0
            for b in range(b0, b0 + g):
                order_bq.append((b, qq))
                nt += int(tiles_bq[b * PIECES + qq])
            batches.append(nt)
            b0 += g
    tot2 = int(tiles_bq.sum())
    assert sum(batches) == tot2 and tot2 > 0

    off_bq = np.zeros(nbq, np.int64)
    acc = 0
    for (b, qq) in order_bq:
        off_bq[b * PIECES + qq] = acc
        acc += int(tiles_bq[b * PIECES + qq])

    per_core = []
    for c in range(NCORES):
        m = c_dst == c
        okey = (blk[m] * PIECES + q[m]).astype(np.int64)
        # secondary sort by source row: gather descriptors within a
        # (block, piece) group read the table in ascending order, which
        # turns random 256B HBM reads into quasi-sequential streams.
        sort = np.lexsort((ric[m], okey))
        okey_s = okey[sort]
        e_rel = rel[m][sort]
        e_ric = ric[m][sort]
        slot_base = off_bq[okey_s] * 128
        grp_start = np.searchsorted(okey_s, okey_s)
        within = np.arange(okey_s.size) - grp_start
        slots = slot_base + within
        idx_flat = np.zeros(tot2 * 128, np.int16)
        rel_flat = np.full(tot2 * 128, -1.0, np.float32)
        idx_flat[slots] = e_ric.astype(np.int16)
        rel_flat[slots] = e_rel
        idx16 = idx_flat.reshape(tot2 * 8, 16).T  # [16, tiles*8]
        idx_plane = np.tile(idx16, (8, 1)).copy()
        rel_plane = np.ascontiguousarray(
            rel_flat.reshape(tot2, 128).T)  # [128, tot2]
        per_core.append(dict(idx_plane=idx_plane, rel2=rel_plane))

    meta2 = dict(tiles_bq=tiles_bq, batches=batches, tot2=tot2,
                 supers=supers)
    return meta2, per_core


def _host_inputs(x, edge_index, W1, b1, W2, b2, cfg):
    d = _derive(cfg)
    bucket, blocks, shard = d["bucket"], d["blocks"], d["shard"]
    n = cfg["N"]
    chid, cout = cfg["CHID"], cfg["COUT"]

    col = edge_index[1].astype(np.int64)
    deg = (np.bincount(col, minlength=n) + 1).astype(np.float32)

    meta1, pc1 = _prep_l1(np.asarray(x, np.float32), edge_index, deg, cfg, d)
    meta2, pc2 = _prep_l2(edge_index, cfg, d)

    w1 = np.ascontiguousarray(np.asarray(W1, np.float32))
    w2 = np.ascontiguousarray(np.asarray(W2, np.float32))
    b1r = np.asarray(b1, np.float32).reshape(1, chid)
    b2r = np.asarray(b2, np.float32).reshape(1, cout)
    iota = np.ascontiguousarray(
        np.tile(np.arange(128, dtype=np.float32)[None, :], (128, 4)))
    eye = np.eye(128, dtype=np.float32)

    in_maps = []
    for c in range(NCORES):
        degs = np.ones(shard, np.float32)
        degs[:bucket] = deg[c * bucket:(c + 1) * bucket]
        deg_pm = np.ascontiguousarray(degs.reshape(blocks, 128).T)
        # [4, (blocks//4)*128]: [k, g*128+p] = deg[(4g+k)*128+p]
        deg_b4 = np.ascontiguousarray(
            degs.reshape(blocks // 4, 4, 128).transpose(1, 0, 2)
            .reshape(4, -1))
        b1sel = np.zeros((4, 4 * chid), np.float32)
        b2sel = np.zeros((4, 4 * cout), np.float32)
        for k in range(4):
            b1sel[k, k * chid:(k + 1) * chid] = b1r[0]
            b2sel[k, k * cout:(k + 1) * cout] = b2r[0]
        in_maps.append({
            "x_exp": pc1[c]["x_exp"], "rel1": pc1[c]["rel1"],
            "deg1": pc1[c]["deg1"],
            "idx_plane": pc2[c]["idx_plane"], "rel2": pc2[c]["rel2"],
            "deg_pm": deg_pm, "deg_b4": deg_b4,
            "w1": w1, "w2": w2, "b1sel": b1sel, "b2sel": b2sel,
            "iota": iota, "eye": eye,
        })
    meta = dict(d=d, m1=meta1, m2=meta2)
    return meta, in_maps


def _build_program(cfg, meta):
    import concourse.bacc as bacc
    import concourse.mybir as mybir
    from concourse import tile

    d = meta["d"]
    blocks, shard, qrows, chunk, bpp = (d["blocks"], d["shard"], d["qrows"],
                                        d["chunk"], d["bpp"])
    supers = d["supers"]
    tiles1 = meta["m1"]["tiles1"]
    off1 = meta["m1"]["off1"]
    tot1 = meta["m1"]["tot1"]
    tiles_bq = meta["m2"]["tiles_bq"]
    tot2 = meta["m2"]["tot2"]
    batches = meta["m2"]["batches"]
    cin, chid, cout = cfg["CIN"], cfg["CHID"], cfg["COUT"]
    nbmax = max(batches)

    bf16 = mybir.dt.bfloat16
    f32 = mybir.dt.float32
    i16 = mybir.dt.int16
    mult = mybir.AluOpType.mult
    amax = mybir.AluOpType.max
    iseq = mybir.AluOpType.is_equal

    nc = bacc.Bacc("TRN2", target_bir_lowering=False, debug=False,
                   num_devices=NCORES, num_swdge_queues=GATHER_QUEUES)

    x_exp_t = nc.dram_tensor("x_exp", [128, tot1 * cin], bf16,
                             kind="ExternalInput")
    rel1_t = nc.dram_tensor("rel1", [128, tot1], f32, kind="ExternalInput")
    deg1_t = nc.dram_tensor("deg1", [128, tot1], f32, kind="ExternalInput")
    idxp_t = nc.dram_tensor("idx_plane", [128, tot2 * 8], i16,
                            kind="ExternalInput")
    rel2_t = nc.dram_tensor("rel2", [128, tot2], f32, kind="ExternalInput")
    deg_pm_t = nc.dram_tensor("deg_pm", [128, blocks], f32,
                              kind="ExternalInput")
    deg_b4_t = nc.dram_tensor("deg_b4", [4, (blocks // 4) * 128], f32,
                              kind="ExternalInput")
    w1_t = nc.dram_tensor("w1", [cin, chid], f32, kind="ExternalInput")
    w2_t = nc.dram_tensor("w2", [chid, cout], f32, kind="ExternalInput")
    b1sel_t = nc.dram_tensor("b1sel", [4, 4 * chid], f32,
                             kind="ExternalInput")
    b2sel_t = nc.dram_tensor("b2sel", [4, 4 * cout], f32,
                             kind="ExternalInput")
    iota_t = nc.dram_tensor("iota", [128, 512], f32, kind="ExternalInput")
    eye_t = nc.dram_tensor("eye", [128, 128], f32, kind="ExternalInput")
    out_t = nc.dram_tensor("out", [shard, cout], f32, kind="ExternalOutput")

    with tile.TileContext(nc) as tc:
        with (
            tc.tile_pool(name="dram", bufs=1, space="DRAM") as dram,
            tc.tile_pool(name="const", bufs=1) as cp,
            tc.tile_pool(name="xstage", bufs=3) as xsp,
            tc.tile_pool(name="stage", bufs=4) as stp,
            tc.tile_pool(name="masks", bufs=12) as mp,
            tc.tile_pool(name="work", bufs=6) as wp,
            tc.tile_pool(name="outp", bufs=4) as op_,
            tc.tile_pool(name="pagg", bufs=4, space="PSUM") as pagg,
            tc.tile_pool(name="pfin", bufs=2, space="PSUM") as pfin,
        ):
            # ---- DRAM scratch ----
            bounce = [dram.tile([qrows, chid], bf16, name=f"bo_{j}",
                                tag=f"bo_{j}") for j in range(PIECES)]
            tab2 = [dram.tile([chunk, chid], bf16, name=f"t2_{j}",
                              tag=f"t2_{j}")
                    for j in range(PIECES)]

            # ---- constants ----
            iota_sb = cp.tile([128, 512], bf16)
            nc.gpsimd.dma_start(iota_sb[:], iota_t[:])  # cast f32->bf16
            iota4 = iota_sb[:].rearrange("p (a b) -> p a b", b=128)
            eye_sb = cp.tile([128, 128], bf16)
            nc.gpsimd.dma_start(eye_sb[:], eye_t[:])
            w1_sb = cp.tile([cin, chid], bf16)
            nc.gpsimd.dma_start(w1_sb[:], w1_t[:])
            w2_sb = cp.tile([chid, cout], bf16)
            nc.gpsimd.dma_start(w2_sb[:], w2_t[:])
            b1_sb = cp.tile([4, 4 * chid], f32)
            nc.sync.dma_start(b1_sb[:], b1sel_t[:])
            b2_sb = cp.tile([4, 4 * cout], f32)
            nc.sync.dma_start(b2_sb[:], b2sel_t[:])
            idxp_sb = cp.tile([128, tot2 * 8], i16)
            nc.sync.dma_start(idxp_sb[:], idxp_t[:])
            rel1_sb = cp.tile([128, tot1], f32)
            nc.sync.dma_start(rel1_sb[:], rel1_t[:])
            deg1_sb = cp.tile([128, tot1], f32)
            nc.sync.dma_start(deg1_sb[:], deg1_t[:])
            rel2_sb = cp.tile([128, tot2], f32)
            nc.sync.dma_start(rel2_sb[:], rel2_t[:])
            deg_pm = cp.tile([128, blocks], f32)
            nc.sync.dma_start(deg_pm[:], deg_pm_t[:])
            deg_b4 = cp.tile([4, (blocks // 4) * 128], f32)
            nc.sync.dma_start(deg_b4[:], deg_b4_t[:])

            # device rsqrt of all degree planes
            sq1 = cp.tile([128, tot1], f32)
            nc.scalar.sqrt(sq1[:], deg1_sb[:])
            dis1 = cp.tile([128, tot1], f32)
            nc.vector.reciprocal(dis1[:], sq1[:])
            invd_pm = cp.tile([128, blocks], f32)
            nc.scalar.sqrt(invd_pm[:], deg_pm[:])
            dis_pm = cp.tile([128, blocks], f32)
            nc.vector.reciprocal(dis_pm[:], invd_pm[:])
            invd_b4 = cp.tile([4, (blocks // 4) * 128], f32)
            nc.scalar.sqrt(invd_b4[:], deg_b4[:])

            # t2 shard rows (this core's bucket, pre-scaled by dis)
            g2s = cp.tile([128, blocks * chid], bf16)
            nc.vector.memset(g2s[:], 0.0)
            zrow = cp.tile([1, 512], f32)
            nc.vector.memset(zrow[:], 0.0)

            def seed_bias(psum, b, bias_sb, w):
                # psum[slot, ch] = sqrt(deg_b)[slot] * bias[ch] via K=4
                # matmul: invd_b4 column group of b with selector row b%4
                nc.tensor.matmul(
                    psum,
                    invd_b4[:, (b // 4) * 128:(b // 4) * 128 + 128],
                    bias_sb[:, (b % 4) * w:(b % 4) * w + w],
                    start=True, stop=False)

            # ========= layer 1 + layer 2 wavefront emission =========
            # L2 (piece, super) work is emitted as soon as its deps exist
            # (g2s blocks of the super + AllGathered table piece), so the
            # PE consumes gather batches while layer 1 is still running
            # and the staging pool keeps recycling for the GpSimd queue.
            part = cp.tile([128, blocks * 128], bf16)
            x3 = x_exp_t[:].rearrange("p (t c) -> p t c", c=cin)
            nsup = len(supers)
            sup_b0 = [sum(supers[:i]) for i in range(nsup)]
            batch_prefix = [0]
            for nb_ in batches:
                batch_prefix.append(batch_prefix[-1] + nb_)
            st_ctr = [0]

            def emit_l2_item(qq, si):
                g = supers[si]
                b0 = sup_b0[si]
                bidx = qq * nsup + si
                cursor = int(batch_prefix[bidx])
                nb = int(batches[bidx])
                banks2 = [pagg.tile([128, 512], f32, tag="bank2",
                                    bufs=4, name="p2b")
                          for _ in range(g // 4)]

                def pacc(bi):
                    return banks2[bi // 4][:, (bi % 4) * 128:
                                           (bi % 4) * 128 + 128]

                # ONE accumulation group per PSUM bank: a bank-wide
                # opener matmul (zeros on pass 0, previous partial
                # re-injected via identity otherwise), one stop/bank.
                seq = []
                if qq == 0:
                    seq += [("self", bi) for bi in range(g)]
                for bi in range(g):
                    ntq = int(tiles_bq[(b0 + bi) * PIECES + qq])
                    for t in range(ntq):
                        seq.append(("edge", bi, t))
                last_per_bank = {k: ("opener", k) for k in range(g // 4)}
                for item in seq:
                    last_per_bank[item[1] // 4] = item
                for k in range(g // 4):
                    stop_k = last_per_bank[k] == ("opener", k)
                    if qq == 0:
                        nc.tensor.matmul(banks2[k][:], zrow[:, 0:128],
                                         zrow[:], start=True, stop=stop_k)
                    else:
                        nc.tensor.matmul(
                            banks2[k][:], eye_sb[:],
                            part[:, (b0 + k * 4) * 128:
                                 (b0 + k * 4 + 4) * 128],
                            start=True, stop=stop_k)
                if qq == 0:
                    # self loops: p2T[ch,slot] += t2_local.T (t2 already
                    # carries one dis; eviction supplies the second)
                    for bi in range(g):
                        b = b0 + bi
                        nc.tensor.matmul(
                            pacc(bi), g2s[:, b * chid:(b + 1) * chid],
                            eye_sb[:], start=False,
                            stop=(last_per_bank[bi // 4] == ("self", bi)))
                if nb > 0:
                    st = stp.tile([128, nbmax, 128], bf16, tag="st",
                                  name="st")
                    if st_ctr[0] < 4:
                        nc.vector.memset(st[:], 0.0)
                    st_ctr[0] += 1
                    nc.gpsimd.dma_gather(
                        st[:, :nb, :], tab2[qq][:],
                        idxp_sb[:, cursor * 8:(cursor + nb) * 8],
                        nb * 128, nb * 128, 128,
                        single_packet=False,
                        queue_num=(st_ctr[0] % GATHER_QUEUES))
                    # flat (bank-slot, tile) sequence; wide mask builds
                    # interleaved per 4 tiles (pure one-hot; dis is
                    # folded into the t2 table)
                    flat = []
                    for bi in range(g):
                        ntq = int(tiles_bq[(b0 + bi) * PIECES + qq])
                        for t in range(ntq):
                            flat.append(
                                (bi, last_per_bank[bi // 4] ==
                                 ("edge", bi, t)))
                    mkg = None
                    for t_local, (bi, stop) in enumerate(flat):
                        if t_local % 4 == 0:
                            gsz = min(4, nb - t_local)
                            mkg = mp.tile([128, 4, 128], bf16,
                                          tag="mk", bufs=6, name="mk2")
                            nc.vector.tensor_tensor(
                                mkg[:, 0:gsz, :], iota4[:, 0:gsz, :],
                                rel2_sb[:, cursor + t_local:
                                        cursor + t_local + gsz]
                                .to_broadcast([128, gsz, 128]), iseq)
                        nc.tensor.matmul(
                            pacc(bi),
                            st[:, t_local:t_local + 1, :].squeeze(),
                            mkg[:, t_local % 4, :].squeeze(),
                            start=False, stop=stop)
                if qq < PIECES - 1:
                    # spill partial aggregate for the next piece pass
                    for bi in range(g):
                        b = b0 + bi
                        nc.scalar.copy(
                            part[:, b * 128:(b + 1) * 128], pacc(bi))
                else:
                    # final: transform + bias + scale, write out
                    fin2 = None
                    for bi in range(g):
                        b = b0 + bi
                        if bi % 4 == 0:
                            fin2 = pfin.tile([128, 512], f32, tag="fin",
                                             name="pob")
                        a2T = wp.tile([128, 128], bf16, tag="aggT",
                                      name="a2T")
                        nc.scalar.copy(a2T[:], pacc(bi))
                        po = fin2[:, (bi % 4) * 128:(bi % 4) * 128 + cout]
                        seed_bias(po, b, b2_sb, cout)
                        nc.tensor.matmul(po, a2T[:], w2_sb[:],
                                         start=False, stop=True)
                        ob = op_.tile([128, cout], f32, tag="ob",
                                      name="ob")
                        nc.scalar.mul(ob[:], po, dis_pm[:, b:b + 1])
                        nc.sync.dma_start(
                            out_t[b * 128:(b + 1) * 128, :], ob[:])

            # ---- layer 1 with interleaved L2 emission ----
            for b4 in range(0, blocks, 4):
                bank = pagg.tile([128, 512], f32, tag="bank1",
                                 bufs=2, name="p1b")
                fin = pfin.tile([128, 512], f32, tag="fin", name="phb")
                for bi in range(4):
                    b = b4 + bi
                    nt = int(tiles1[b])
                    t0 = int(off1[b])
                    if nt > 0:
                        p1 = bank[:, bi * 128:bi * 128 + 128]
                        xs = xsp.tile([128, nt, cin], bf16, tag="xs",
                                      name="xs")
                        nc.sync.dma_start(xs[:], x3[:, t0:t0 + nt, :])
                        for gc in range(0, nt, 4):
                            gsz = min(4, nt - gc)
                            mkg = mp.tile([128, 4, 128], bf16,
                                          tag="mk1", bufs=8, name="mk1")
                            nc.vector.tensor_tensor(
                                mkg[:, 0:gsz, :], iota4[:, 0:gsz, :],
                                rel1_sb[:, t0 + gc:t0 + gc + gsz]
                                .to_broadcast([128, gsz, 128]), iseq)
                            nc.vector.tensor_tensor(
                                mkg[:, 0:gsz, :], mkg[:, 0:gsz, :],
                                dis1[:, t0 + gc:t0 + gc + gsz]
                                .to_broadcast([128, gsz, 128]), mult)
                            for j in range(gsz):
                                t = gc + j
                                nc.tensor.matmul(
                                    p1, xs[:, t, :].squeeze(),
                                    mkg[:, j, :].squeeze(),
                                    start=(t == 0), stop=(t == nt - 1))
                        aggT = wp.tile([128, 128], bf16, tag="aggT",
                                       name="aggT")
                        nc.scalar.copy(aggT[:], p1)
                        ph = fin[:, bi * 128:bi * 128 + chid]
                        seed_bias(ph, b, b1_sb, chid)
                        nc.tensor.matmul(ph, aggT[:], w1_sb[:],
                                         start=False, stop=True)
                        h1r = wp.tile([128, chid], bf16, tag="h1r",
                                      name="h1r")
                        nc.scalar.activation(
                            h1r[:], ph, mybir.ActivationFunctionType.Relu,
                            bias=0.0, scale=dis_pm[:, b:b + 1])
                        nc.scalar.mul(
                            g2s[:, b * chid:(b + 1) * chid], h1r[:],
                            dis_pm[:, b:b + 1])
                    # piece boundary: export, AllGather, release L2 work
                    if (b + 1) % bpp == 0:
                        j = b // bpp
                        g2s3 = g2s[:].rearrange("p (bb c) -> p bb c",
                                                c=chid)
                        nc.sync.dma_start(
                            bounce[j][:].rearrange("(bb p) c -> p bb c",
                                                   p=128),
                            g2s3[:, j * bpp:(j + 1) * bpp, :])

            if DBG_L1_OUT:
                for b in range(blocks):
                    ob = op_.tile([128, cout], f32, tag="ob", name="obd")
                    nc.vector.tensor_copy(
                        ob[:], g2s[:, b * chid:b * chid + cout])
                    nc.sync.dma_start(out_t[b * 128:(b + 1) * 128, :],
                                      ob[:])

            # ---- layer 2: piece-major passes (AllGather at pass start) ----
            for qq in (range(PIECES) if not DBG_L1_OUT else []):
                if DBG_NO_COLL:
                    nc.sync.dma_start(tab2[qq][0:qrows, :], bounce[qq][:])
                else:
                    nc.gpsimd.collective_compute(
                        "AllGather", mybir.AluOpType.bypass,
                        replica_groups=[list(range(NCORES))],
                        ins=[bounce[qq].opt()], outs=[tab2[qq].opt()])
                for si in range(nsup):
                    emit_l2_item(qq, si)

    nc.compile()
    return nc


def run_config(inputs, cfg, run=None):
    from concourse.bass_utils import run_bass_kernel_spmd

    x = np.asarray(inputs["x"], np.float32)
    edge_index = np.asarray(inputs["edge_index"])
    meta, in_maps = _host_inputs(
        x, edge_index, inputs["W1"], inputs["b1"], inputs["W2"],
        inputs["b2"], cfg)
    nc = _build_program(cfg, meta)
    if run is None:
        def run(nc, in_maps):
            return run_bass_kernel_spmd(
                nc, in_maps, list(range(NCORES))).results
    results = run(nc, in_maps)
    bucket = _derive(cfg)["bucket"]
    out = np.concatenate(
        [results[c]["out"][:bucket] for c in range(NCORES)], axis=0)
    return np.ascontiguousarray(out.astype(np.float32))


def kernel(**inputs):
    return run_config(inputs, CFG_FULL)

